# revision 11
# baseline (speedup 1.0000x reference)
"""Causal multi-head attention on 8 trn2 NeuronCores.

Sharding: core c handles batch b=c//4 and heads [4*(c%4), 4*(c%4)+4).
Each core computes its 4 heads' attention plus the partial output
projection against the matching 256 rows of Wo; the host sums the 4
partials per batch (the all-reduce implied by row-sharding Wo) and adds
bo.

v5 (all matmul operands bf16, PSUM accumulation fp32):
  - bf16 matmuls: the fp32r path runs in fp32_mode=HIGH at ~3.5
    cycles/col on HW; bf16 runs at 1 cycle/col and halves LDWEIGHTS.
  - K bias dropped (softmax shift-invariance: only bq.k survives among
    the bias cross terms); Q bias rides the ACT eviction; V bias rides
    the DVE eviction (tensor_add vs a partition-broadcast tile) - zero
    extra matmuls for biases. 1/sqrt(dk) folded into Wq/bq on the host.
  - Causal mask added in PSUM via identity-weight matmul of a bf16 -1e9
    tile (cheaper than a DVE pass and keeps the exp->ctx chain short).
  - The ctx matmul trails one chunk behind exp so the PE in-order queue
    has score+mask work in flight while ACT computes exp.
  - Softmax denominator rides a ones column in the Vaug lhsT; the
    normalization is denominator row -> bf16 SBUF, rank-1 PE broadcast
    to all 128 partitions (custom-DVE ops misbehave based at partition
    64, so everything stays base-0), reciprocal_approx_fast + multiply
    on DVE.
  - Input DMA split across two queues (xt on the gpsimd SWDGE, weights
    on sync) with chunk 0 first so the first matmuls start early.
"""

import sys

for _p in ("/opt/trn_rl_repo", "/root/.axon_site/_ro/trn_rl_repo"):
    if _p not in sys.path:
        sys.path.insert(0, _p)

import numpy as np

import concourse.bass as bass
import concourse.bacc as bacc
import concourse.tile as tile
from concourse import mybir
from concourse.bass_utils import run_bass_kernel_spmd

F32 = mybir.dt.float32
BF16 = mybir.dt.bfloat16

B, S, D, H, DK = 2, 2048, 1024, 16, 64
NCORES = 8
HPC = 4          # heads per core
NPAIR = 2        # head pairs per core
ND = D // 128    # 8 contraction chunks over d
NS = S // 512    # 4 query blocks
NS16 = S // 128  # 16 sequence chunks

_CACHE = {}


def _build_bass():
    nc = bacc.Bacc(None)
    xt = nc.dram_tensor("xt", [D, S], BF16, kind="ExternalInput")
    wq = nc.dram_tensor("wq", [NPAIR, D, 128], BF16, kind="ExternalInput")
    wk = nc.dram_tensor("wk", [NPAIR, D, 128], BF16, kind="ExternalInput")
    wv = nc.dram_tensor("wv", [D, 256], BF16, kind="ExternalInput")
    wo = nc.dram_tensor("wo", [256, D], BF16, kind="ExternalInput")
    bq = nc.dram_tensor("bq", [128, NPAIR], F32, kind="ExternalInput")
    bv_bc = nc.dram_tensor("bv_bc", [128, NPAIR, 2, 64], F32, kind="ExternalInput")
    mneg = nc.dram_tensor("mneg", [128, 4, 512], BF16, kind="ExternalInput")
    ident = nc.dram_tensor("ident", [128, 128], BF16, kind="ExternalInput")
    ones = nc.dram_tensor("ones", [1, 128], BF16, kind="ExternalInput")
    vfix = nc.dram_tensor("vfix", [128, 64], BF16, kind="ExternalInput")
    out = nc.dram_tensor("out", [S, D], F32, kind="ExternalOutput")

    with nc.allow_low_precision("bf16 operands; accumulation stays fp32 in PSUM"), \
            tile.TileContext(nc) as tc:
        with (
            tc.tile_pool(name="consts", bufs=1) as consts,
            tc.tile_pool(name="qkv", bufs=1) as qkv,
        ):
            wq_sb = consts.tile([128, NPAIR, ND, 128], BF16, tag="wq")
            wk_sb = consts.tile([128, NPAIR, ND, 128], BF16, tag="wk")
            wv_sb = consts.tile([128, ND, 256], BF16, tag="wv")
            wo_sb = consts.tile([128, 2, D], BF16, tag="wo")
            bq_sb = consts.tile([128, NPAIR], F32, tag="bq")
            bv_sb = consts.tile([128, NPAIR, 2, 64], F32, tag="bv")
            mneg_sb = consts.tile([128, 4, 512], BF16, tag="mneg")
            ident_sb = consts.tile([128, 128], BF16, tag="ident")
            ones_sb = consts.tile([1, 128], BF16, tag="ones")

            qt_sb = qkv.tile([128, NPAIR, S], BF16, tag="qt")
            kt_sb = qkv.tile([128, NPAIR, S], BF16, tag="kt")
            # Vaug per pair: cols 0:64 V_even | 64 ones | 65:128 zeros
            # | 128:192 V_odd. Even lhsT = cols 0:65 -> ctx on parts
            # 0:64 (+denominator row 64); odd lhsT = cols 64:192 ->
            # denominator on part 0, ctx on parts 64:128.
            va_sb = qkv.tile([128, NPAIR, NS16, 192], BF16, tag="va")
            ctxcat_sb = qkv.tile([128, 2, S], BF16, tag="ctxcat")

            with (
                tc.tile_pool(name="xp", bufs=1) as xp,
                tc.tile_pool(name="mmp", bufs=8, space="PSUM") as mmp,
            ):
                xt_sb = xp.tile([128, ND, S], BF16, tag="xt")
                # xt on the gpsimd SWDGE queue, weights/consts on sync:
                # the two queues transfer in parallel. Chunk 0 first so
                # the first projection matmuls start early.
                nc.sync.dma_start(out=xt_sb[:, 0, :], in_=xt[0:128, :])
                for p in range(NPAIR):
                    nc.sync.dma_start(out=wq_sb[:, p, 0, :], in_=wq[p, 0:128, :])
                    nc.sync.dma_start(out=wk_sb[:, p, 0, :], in_=wk[p, 0:128, :])
                nc.sync.dma_start(out=wv_sb[:, 0, :], in_=wv[0:128, :])
                nc.sync.dma_start(out=bq_sb[:], in_=bq[:])
                nc.sync.dma_start(out=bv_sb[:], in_=bv_bc[:])
                nc.sync.dma_start(out=ident_sb[:], in_=ident[:])
                nc.sync.dma_start(out=ones_sb[:], in_=ones[:])
                for p in range(NPAIR):
                    vfix_bc = bass.AP(
                        tensor=vfix.ap().tensor,
                        offset=0,
                        ap=[[64, 128], [0, NS16], [1, 64]],
                    )
                    nc.gpsimd.dma_start(out=va_sb[:, p, :, 64:128], in_=vfix_bc)
                for c in range(1, ND):
                    nc.gpsimd.dma_start(
                        out=xt_sb[:, c, :], in_=xt[c * 128:(c + 1) * 128, :]
                    )
                    for p in range(NPAIR):
                        nc.sync.dma_start(
                            out=wq_sb[:, p, c, :],
                            in_=wq[p, c * 128:(c + 1) * 128, :],
                        )
                        nc.sync.dma_start(
                            out=wk_sb[:, p, c, :],
                            in_=wk[p, c * 128:(c + 1) * 128, :],
                        )
                    nc.sync.dma_start(
                        out=wv_sb[:, c, :], in_=wv[c * 128:(c + 1) * 128, :]
                    )
                nc.sync.dma_start(out=mneg_sb[:], in_=mneg[:])
                for k in range(2):
                    nc.sync.dma_start(
                        out=wo_sb[:, k, :], in_=wo[k * 128:(k + 1) * 128, :]
                    )

                # ---- Q^T / K^T projections (per pair, dk on partitions)
                for p in range(NPAIR):
                    for sb in range(NS):
                        qp = mmp.tile([128, 512], F32, tag="mm", name="qp")
                        for c in range(ND):
                            nc.tensor.matmul(
                                qp[:],
                                lhsT=wq_sb[:, p, c, :],
                                rhs=xt_sb[:, c, sb * 512:(sb + 1) * 512],
                                start=(c == 0),
                                stop=(c == ND - 1),
                            )
                        nc.scalar.activation(
                            out=qt_sb[:, p, sb * 512:(sb + 1) * 512],
                            in_=qp[:],
                            func=mybir.ActivationFunctionType.Identity,
                            bias=bq_sb[:, p:p + 1],
                            scale=1.0,
                        )
                        kp = mmp.tile([128, 512], F32, tag="mm", name="kp")
                        for c in range(ND):
                            nc.tensor.matmul(
                                kp[:],
                                lhsT=wk_sb[:, p, c, :],
                                rhs=xt_sb[:, c, sb * 512:(sb + 1) * 512],
                                start=(c == 0),
                                stop=(c == ND - 1),
                            )
                        nc.vector.tensor_copy(
                            out=kt_sb[:, p, sb * 512:(sb + 1) * 512],
                            in_=kp[:],
                        )

                # ---- V in natural layout [s, dk], 4 heads at once.
                # bv is added during the eviction (tensor_add with a
                # partition-broadcast constant): exact through the softmax
                # denominator trick since rows of P sum to den.
                for s16 in range(NS16):
                    vp = mmp.tile([128, 256], F32, tag="mm", name="vp")
                    for c in range(ND):
                        nc.tensor.matmul(
                            vp[:],
                            lhsT=xt_sb[:, c, s16 * 128:(s16 + 1) * 128],
                            rhs=wv_sb[:, c, :],
                            start=(c == 0),
                            stop=(c == ND - 1),
                        )
                    # V_even -> va cols 0:64, V_odd -> cols 128:192 in one
                    # two-segment add per pair
                    for p in range(NPAIR):
                        d0 = va_sb[:, p, s16, 0:64]
                        dst = bass.AP(
                            tensor=d0.tensor, offset=d0.offset,
                            ap=[[d0.ap[0][0], 128], [128, 2], [1, 64]],
                        )
                        s0 = vp[:, p * 128:(p + 1) * 128]
                        src = bass.AP(
                            tensor=s0.tensor, offset=s0.offset,
                            ap=[[s0.ap[0][0], 128], [64, 2], [1, 64]],
                        )
                        nc.vector.tensor_add(
                            out=dst, in0=src, in1=bv_sb[:, p, :, :]
                        )

            # ---- attention + output projection, per query block
            with (
                tc.tile_pool(name="stp", bufs=4, space="PSUM") as stp,
                tc.tile_pool(name="ctxp", bufs=2, space="PSUM") as ctxp,
                tc.tile_pool(name="ptp", bufs=8) as ptp,
                tc.tile_pool(name="smp", bufs=3) as smp,
                tc.tile_pool(name="outp", bufs=3) as outp,
            ):
                def emit_norm(ctx_ps, even, p, qb, h):
                    # normalization, partition-aligned per parity.
                    # Emitted one head late so the PE stream has score/ctx
                    # work in flight while DVE/PE turn the denominator
                    # into a broadcast reciprocal.
                    cs = 64 if even else 0
                    lo = 0 if even else 64
                    den = smp.tile([1, 512], BF16, tag="den", name="den")
                    nc.vector.tensor_copy(out=den[:], in_=ctx_ps[cs:cs + 1, :])
                    # broadcast to all 128 partitions: custom-DVE ops (and
                    # tile_position=(0,64) matmuls) misbehave on HW when
                    # based at partition 64, so keep everything at base 0.
                    bc_ps = stp.tile([128, 512], F32, tag="st", name="bc_ps")
                    nc.tensor.matmul(
                        bc_ps[:],
                        lhsT=ones_sb[0:1, :],
                        rhs=den[:],
                        start=True,
                        stop=True,
                    )
                    rcp = smp.tile([128, 512], F32, tag="rcp", name="rcp")
                    nc.vector.reciprocal_approx_fast(
                        out=rcp[:], in_=bc_ps[:]
                    )
                    nc.vector.tensor_mul(
                        out=ctxcat_sb[lo:lo + 64, p, qb * 512:(qb + 1) * 512],
                        in0=ctx_ps[lo:lo + 64, :],
                        in1=rcp[lo:lo + 64, :],
                    )

                def emit_outproj(qb):
                    for s16 in range(qb * 4, (qb + 1) * 4):
                        for do in range(2):
                            op = ctxp.tile([128, 512], F32, tag="op", name="op", bufs=2)
                            nc.tensor.matmul(
                                op[:],
                                lhsT=ctxcat_sb[:, 0, s16 * 128:(s16 + 1) * 128],
                                rhs=wo_sb[:, 0, do * 512:(do + 1) * 512],
                                start=True,
                                stop=False,
                            )
                            nc.tensor.matmul(
                                op[:],
                                lhsT=ctxcat_sb[:, 1, s16 * 128:(s16 + 1) * 128],
                                rhs=wo_sb[:, 1, do * 512:(do + 1) * 512],
                                start=False,
                                stop=True,
                            )
                            ot = outp.tile([128, 512], F32, tag="ot", name="ot")
                            if do == 0:
                                nc.scalar.copy(out=ot[:], in_=op[:])
                            else:
                                nc.vector.tensor_copy(out=ot[:], in_=op[:])
                            nc.sync.dma_start(
                                out=out[s16 * 128:(s16 + 1) * 128,
                                        do * 512:(do + 1) * 512],
                                in_=ot[:],
                            )

                pending = None
                for qb in range(NS):
                    nch = (qb + 1) * 4
                    for h in range(HPC):
                        p, j = h // 2, h % 2
                        even = j == 0
                        qs = qt_sb[j * 64:(j + 1) * 64, p, qb * 512:(qb + 1) * 512]
                        ctx_ps = ctxp.tile([128, 512], F32, tag="ctx", name="ctx_ps")
                        ctx_out = ctx_ps[0:65, :] if even else ctx_ps[:]
                        lagged = []  # (pt, f0, diag, c) awaiting their ctx mms

                        def emit_ctx(lag):
                            pt, f0, diag, c = lag
                            lhsT_v = (
                                va_sb[:, p, c, 0:65]
                                if even
                                else va_sb[:, p, c, 64:192]
                            )
                            nc.tensor.matmul(
                                ctx_out[:, f0:512] if diag else ctx_out,
                                lhsT=lhsT_v,
                                rhs=pt[:, f0:512],
                                start=(c == 0),
                                stop=(c == nch - 1),
                            )

                        for c in range(nch):
                            st = stp.tile([128, 512], F32, tag="st", name="st")
                            diag = c >= qb * 4
                            # columns [0, f0) of this block are fully masked
                            # (q < kv for all partitions): skip them entirely.
                            f0 = 128 * (c - qb * 4) if diag else 0
                            nc.tensor.matmul(
                                st[:, f0:512],
                                lhsT=kt_sb[j * 64:(j + 1) * 64, p,
                                           c * 128:(c + 1) * 128],
                                rhs=qs[:, f0:512],
                                start=True,
                                stop=not diag,
                            )
                            if diag:
                                nc.tensor.matmul(
                                    st[:, f0:512],
                                    lhsT=ident_sb[:],
                                    rhs=mneg_sb[:, c - qb * 4, f0:512],
                                    start=False,
                                    stop=True,
                                )
                            pt = ptp.tile([128, 512], BF16, tag="pt", name="pt")
                            nc.scalar.activation(
                                out=pt[:, f0:512],
                                in_=st[:, f0:512],
                                func=mybir.ActivationFunctionType.Exp,
                            )
                            # ctx trails TWO chunks behind its exp so the PE
                            # queue stays deep (hides ACT latency, semaphore
                            # propagation, and LDWEIGHTS of the next matmul).
                            lagged.append((pt, f0, diag, c))
                            if len(lagged) > 3:
                                emit_ctx(lagged.pop(0))
                        for lag in lagged:
                            emit_ctx(lag)

                        if pending is not None:
                            emit_norm(*pending)
                        pending = (ctx_ps, even, p, qb, h)
                        if h == 0 and qb > 0:
                            emit_outproj(qb - 1)

                emit_norm(*pending)
                pending = None
                emit_outproj(NS - 1)
    if not nc.is_finalized():
        nc.finalize()
    return nc


def _prep_inputs(embeddings, Wq, bq, Wk, bk, Wv, bv, Wo, bo):
    embeddings = np.asarray(embeddings, np.float32)
    Wq, bq = np.asarray(Wq, np.float32), np.asarray(bq, np.float32)
    Wk = np.asarray(Wk, np.float32)
    Wv, bv = np.asarray(Wv, np.float32), np.asarray(bv, np.float32)
    Wo = np.asarray(Wo, np.float32)

    import ml_dtypes
    bf16_t = ml_dtypes.bfloat16
    p_idx = np.arange(128)
    mneg = np.zeros((128, 4, 512), np.float32)
    for i in range(4):
        f = np.arange(512)[None, :]
        mneg[:, i, :] = np.where(f >= p_idx[:, None] + 128 * i, 0.0, -1e9)
    mneg = mneg.astype(bf16_t)
    ident = np.eye(128, dtype=np.float32).astype(bf16_t)
    vfix = np.zeros((128, 64), np.float32)
    vfix[:, 0] = 1.0
    vfix = vfix.astype(bf16_t)
    ones = np.ones((1, 128), np.float32).astype(bf16_t)

    in_maps = []
    for c in range(NCORES):
        b, g = c // 4, c % 4
        hs = HPC * g
        xt = np.ascontiguousarray(embeddings[b].T).astype(bf16_t)
        # 1/sqrt(dk) folded into Wq/bq (exact power of two)
        wq2 = np.stack(
            [np.concatenate([Wq[hs + 2 * p], Wq[hs + 2 * p + 1]], axis=1)
             for p in range(NPAIR)]
        ) * 0.125
        wk2 = np.stack(
            [np.concatenate([Wk[hs + 2 * p], Wk[hs + 2 * p + 1]], axis=1)
             for p in range(NPAIR)]
        )
        wv4 = np.concatenate([Wv[hs + h] for h in range(HPC)], axis=1)
        wo4 = np.ascontiguousarray(Wo[hs * DK:(hs + HPC) * DK, :]).astype(bf16_t)
        bq2 = np.stack(
            [np.concatenate([bq[hs + 2 * p], bq[hs + 2 * p + 1]]) / 8.0
             for p in range(NPAIR)], axis=1
        )
        bvb = np.zeros((128, NPAIR, 2, 64), np.float32)
        for p in range(NPAIR):
            bvb[:, p, 0, :] = bv[hs + 2 * p][None, :]
            bvb[:, p, 1, :] = bv[hs + 2 * p + 1][None, :]
        in_maps.append({
            "xt": xt,
            "wq": np.ascontiguousarray(wq2).astype(bf16_t),
            "wk": np.ascontiguousarray(wk2).astype(bf16_t),
            "wv": np.ascontiguousarray(wv4).astype(bf16_t),
            "wo": wo4,
            "bq": np.ascontiguousarray(bq2),
            "bv_bc": bvb,
            "mneg": mneg,
            "ident": ident,
            "ones": ones,
            "vfix": vfix,
        })
    return in_maps


def kernel(embeddings, Wq, bq, Wk, bk, Wv, bv, Wo, bo, _trace=False, _trace_kw=None):
    if "nc" not in _CACHE:
        _CACHE["nc"] = _build_bass()
    nc = _CACHE["nc"]
    in_maps = _prep_inputs(embeddings, Wq, bq, Wk, bk, Wv, bv, Wo, bo)
    kw = dict(_trace_kw or {})
    res = run_bass_kernel_spmd(
        nc, in_maps, core_ids=list(range(NCORES)), trace=_trace, **kw
    )
    _CACHE["last_result"] = res
    bo32 = np.asarray(bo, np.float32)
    out = np.empty((B, S, D), np.float32)
    for b in range(B):
        acc = np.array(res.results[4 * b]["out"], np.float32, copy=True)
        for g in range(1, 4):
            acc += np.asarray(res.results[4 * b + g]["out"], np.float32)
        out[b] = acc + bo32
    return out


# revision 12
# speedup vs baseline: 1.0744x; 1.0744x over previous
"""Causal multi-head attention on 8 trn2 NeuronCores.

Sharding: core c handles batch b=c//4 and heads [4*(c%4), 4*(c%4)+4).
Each core computes its 4 heads' attention plus the partial output
projection against the matching 256 rows of Wo; the host sums the 4
partials per batch (the all-reduce implied by row-sharding Wo) and adds
bo.

v5 (all matmul operands bf16, PSUM accumulation fp32):
  - bf16 matmuls: the fp32r path runs in fp32_mode=HIGH at ~3.5
    cycles/col on HW; bf16 runs at 1 cycle/col and halves LDWEIGHTS.
  - K bias dropped (softmax shift-invariance: only bq.k survives among
    the bias cross terms); Q bias rides the ACT eviction; V bias rides
    the DVE eviction (tensor_add vs a partition-broadcast tile) - zero
    extra matmuls for biases. 1/sqrt(dk) folded into Wq/bq on the host.
  - Causal mask added in PSUM via identity-weight matmul of a bf16 -1e9
    tile (cheaper than a DVE pass and keeps the exp->ctx chain short).
  - The ctx matmul trails one chunk behind exp so the PE in-order queue
    has score+mask work in flight while ACT computes exp.
  - Softmax denominator rides a ones column in the Vaug lhsT; the
    normalization is denominator row -> bf16 SBUF, rank-1 PE broadcast
    to all 128 partitions (custom-DVE ops misbehave based at partition
    64, so everything stays base-0), reciprocal_approx_fast + multiply
    on DVE.
  - Input DMA split across two queues (xt on the gpsimd SWDGE, weights
    on sync) with chunk 0 first so the first matmuls start early.
"""

import sys

for _p in ("/opt/trn_rl_repo", "/root/.axon_site/_ro/trn_rl_repo"):
    if _p not in sys.path:
        sys.path.insert(0, _p)

import numpy as np

import concourse.bass as bass
import concourse.bacc as bacc
import concourse.tile as tile
from concourse import mybir
from concourse.bass_utils import run_bass_kernel_spmd

F32 = mybir.dt.float32
BF16 = mybir.dt.bfloat16

B, S, D, H, DK = 2, 2048, 1024, 16, 64
NCORES = 8
HPC = 4          # heads per core
NPAIR = 2        # head pairs per core
ND = D // 128    # 8 contraction chunks over d
NS = S // 512    # 4 query blocks
NS16 = S // 128  # 16 sequence chunks

_CACHE = {}


def _build_bass():
    nc = bacc.Bacc(None)
    xt = nc.dram_tensor("xt", [D, S], BF16, kind="ExternalInput")
    wq = nc.dram_tensor("wq", [NPAIR, D, 128], BF16, kind="ExternalInput")
    wk = nc.dram_tensor("wk", [NPAIR, D, 128], BF16, kind="ExternalInput")
    wv = nc.dram_tensor("wv", [D, 256], BF16, kind="ExternalInput")
    wo = nc.dram_tensor("wo", [256, D], BF16, kind="ExternalInput")
    bq = nc.dram_tensor("bq", [128, NPAIR], F32, kind="ExternalInput")
    bv_bc = nc.dram_tensor("bv_bc", [128, NPAIR, 2, 64], F32, kind="ExternalInput")
    mneg = nc.dram_tensor("mneg", [128, 4, 512], BF16, kind="ExternalInput")
    ident = nc.dram_tensor("ident", [128, 128], BF16, kind="ExternalInput")
    ones = nc.dram_tensor("ones", [1, 128], BF16, kind="ExternalInput")
    vfix = nc.dram_tensor("vfix", [128, 64], BF16, kind="ExternalInput")
    out = nc.dram_tensor("out", [S, D], F32, kind="ExternalOutput")

    with nc.allow_low_precision("bf16 operands; accumulation stays fp32 in PSUM"), \
            tile.TileContext(nc) as tc:
        with (
            tc.tile_pool(name="consts", bufs=1) as consts,
            tc.tile_pool(name="qkv", bufs=1) as qkv,
        ):
            wq_sb = consts.tile([128, NPAIR, ND, 128], BF16, tag="wq")
            wk_sb = consts.tile([128, NPAIR, ND, 128], BF16, tag="wk")
            wv_sb = consts.tile([128, ND, 256], BF16, tag="wv")
            wo_sb = consts.tile([128, 2, D], BF16, tag="wo")
            bq_sb = consts.tile([128, NPAIR], F32, tag="bq")
            bv_sb = consts.tile([128, NPAIR, 2, 64], F32, tag="bv")
            mneg_sb = consts.tile([128, 4, 512], BF16, tag="mneg")
            ident_sb = consts.tile([128, 128], BF16, tag="ident")
            ones_sb = consts.tile([1, 128], BF16, tag="ones")

            qt_sb = qkv.tile([128, NPAIR, S], BF16, tag="qt")
            kt_sb = qkv.tile([128, NPAIR, S], BF16, tag="kt")
            # Vaug per pair: cols 0:64 V_even | 64 ones | 65:128 zeros
            # | 128:192 V_odd. Even lhsT = cols 0:65 -> ctx on parts
            # 0:64 (+denominator row 64); odd lhsT = cols 64:192 ->
            # denominator on part 0, ctx on parts 64:128.
            va_sb = qkv.tile([128, NPAIR, NS16, 192], BF16, tag="va")
            ctxcat_sb = qkv.tile([128, 2, S], BF16, tag="ctxcat")

            with (
                tc.tile_pool(name="xp", bufs=1) as xp,
                tc.tile_pool(name="mmp", bufs=8, space="PSUM") as mmp,
            ):
                xt_sb = xp.tile([128, ND, S], BF16, tag="xt")
                # xt on the gpsimd SWDGE queue, weights/consts on sync:
                # the two queues transfer in parallel. Chunk 0 first so
                # the first projection matmuls start early.
                nc.gpsimd.dma_start(out=xt_sb[:, 0, :], in_=xt[0:128, :])
                for p in range(NPAIR):
                    nc.sync.dma_start(out=wq_sb[:, p, 0, :], in_=wq[p, 0:128, :])
                    nc.sync.dma_start(out=wk_sb[:, p, 0, :], in_=wk[p, 0:128, :])
                nc.sync.dma_start(out=wv_sb[:, 0, :], in_=wv[0:128, :])
                nc.sync.dma_start(out=bq_sb[:], in_=bq[:])
                nc.sync.dma_start(out=bv_sb[:], in_=bv_bc[:])
                nc.sync.dma_start(out=ident_sb[:], in_=ident[:])
                nc.sync.dma_start(out=ones_sb[:], in_=ones[:])
                for p in range(NPAIR):
                    vfix_bc = bass.AP(
                        tensor=vfix.ap().tensor,
                        offset=0,
                        ap=[[64, 128], [0, NS16], [1, 64]],
                    )
                    nc.gpsimd.dma_start(out=va_sb[:, p, :, 64:128], in_=vfix_bc)
                for c in range(1, ND):
                    nc.gpsimd.dma_start(
                        out=xt_sb[:, c, :], in_=xt[c * 128:(c + 1) * 128, :]
                    )
                    for p in range(NPAIR):
                        nc.sync.dma_start(
                            out=wq_sb[:, p, c, :],
                            in_=wq[p, c * 128:(c + 1) * 128, :],
                        )
                        nc.sync.dma_start(
                            out=wk_sb[:, p, c, :],
                            in_=wk[p, c * 128:(c + 1) * 128, :],
                        )
                    nc.sync.dma_start(
                        out=wv_sb[:, c, :], in_=wv[c * 128:(c + 1) * 128, :]
                    )
                nc.sync.dma_start(out=mneg_sb[:], in_=mneg[:])
                for k in range(2):
                    nc.sync.dma_start(
                        out=wo_sb[:, k, :], in_=wo[k * 128:(k + 1) * 128, :]
                    )

                # ---- Q^T / K^T projections (per pair, dk on partitions)
                for p in range(NPAIR):
                    for sb in range(NS):
                        qp = mmp.tile([128, 512], F32, tag="mm", name="qp")
                        for c in range(ND):
                            nc.tensor.matmul(
                                qp[:],
                                lhsT=wq_sb[:, p, c, :],
                                rhs=xt_sb[:, c, sb * 512:(sb + 1) * 512],
                                start=(c == 0),
                                stop=(c == ND - 1),
                            )
                        nc.scalar.activation(
                            out=qt_sb[:, p, sb * 512:(sb + 1) * 512],
                            in_=qp[:],
                            func=mybir.ActivationFunctionType.Identity,
                            bias=bq_sb[:, p:p + 1],
                            scale=1.0,
                        )
                        kp = mmp.tile([128, 512], F32, tag="mm", name="kp")
                        for c in range(ND):
                            nc.tensor.matmul(
                                kp[:],
                                lhsT=wk_sb[:, p, c, :],
                                rhs=xt_sb[:, c, sb * 512:(sb + 1) * 512],
                                start=(c == 0),
                                stop=(c == ND - 1),
                            )
                        nc.vector.tensor_copy(
                            out=kt_sb[:, p, sb * 512:(sb + 1) * 512],
                            in_=kp[:],
                        )

                # ---- V in natural layout [s, dk], 4 heads at once.
                # bv is added during the eviction (tensor_add with a
                # partition-broadcast constant): exact through the softmax
                # denominator trick since rows of P sum to den.
                for s16 in range(NS16):
                    vp = mmp.tile([128, 256], F32, tag="mm", name="vp")
                    for c in range(ND):
                        nc.tensor.matmul(
                            vp[:],
                            lhsT=xt_sb[:, c, s16 * 128:(s16 + 1) * 128],
                            rhs=wv_sb[:, c, :],
                            start=(c == 0),
                            stop=(c == ND - 1),
                        )
                    # V_even -> va cols 0:64, V_odd -> cols 128:192 in one
                    # two-segment add per pair
                    for p in range(NPAIR):
                        d0 = va_sb[:, p, s16, 0:64]
                        dst = bass.AP(
                            tensor=d0.tensor, offset=d0.offset,
                            ap=[[d0.ap[0][0], 128], [128, 2], [1, 64]],
                        )
                        s0 = vp[:, p * 128:(p + 1) * 128]
                        src = bass.AP(
                            tensor=s0.tensor, offset=s0.offset,
                            ap=[[s0.ap[0][0], 128], [64, 2], [1, 64]],
                        )
                        nc.vector.tensor_add(
                            out=dst, in0=src, in1=bv_sb[:, p, :, :]
                        )

            # ---- attention + output projection, per query block
            with (
                tc.tile_pool(name="stp", bufs=4, space="PSUM") as stp,
                tc.tile_pool(name="ctxp", bufs=2, space="PSUM") as ctxp,
                tc.tile_pool(name="ptp", bufs=6) as ptp,
                tc.tile_pool(name="smp", bufs=3) as smp,
                tc.tile_pool(name="outp", bufs=3) as outp,
            ):
                def emit_norm(ctx_ps, even, p, qb, h):
                    # normalization, partition-aligned per parity.
                    # Emitted one head late so the PE stream has score/ctx
                    # work in flight while DVE/PE turn the denominator
                    # into a broadcast reciprocal.
                    cs = 64 if even else 0
                    lo = 0 if even else 64
                    den = smp.tile([1, 512], BF16, tag="den", name="den")
                    nc.vector.tensor_copy(out=den[:], in_=ctx_ps[cs:cs + 1, :])
                    # broadcast to all 128 partitions: custom-DVE ops (and
                    # tile_position=(0,64) matmuls) misbehave on HW when
                    # based at partition 64, so keep everything at base 0.
                    bc_ps = stp.tile([128, 512], F32, tag="st", name="bc_ps")
                    nc.tensor.matmul(
                        bc_ps[:],
                        lhsT=ones_sb[0:1, :],
                        rhs=den[:],
                        start=True,
                        stop=True,
                    )
                    rcp = smp.tile([128, 512], F32, tag="rcp", name="rcp")
                    nc.vector.reciprocal_approx_fast(
                        out=rcp[:], in_=bc_ps[:]
                    )
                    nc.vector.tensor_mul(
                        out=ctxcat_sb[lo:lo + 64, p, qb * 512:(qb + 1) * 512],
                        in0=ctx_ps[lo:lo + 64, :],
                        in1=rcp[lo:lo + 64, :],
                    )

                def emit_outproj(qb):
                    for s16 in range(qb * 4, (qb + 1) * 4):
                        for do in range(2):
                            op = ctxp.tile([128, 512], F32, tag="op", name="op", bufs=2)
                            nc.tensor.matmul(
                                op[:],
                                lhsT=ctxcat_sb[:, 0, s16 * 128:(s16 + 1) * 128],
                                rhs=wo_sb[:, 0, do * 512:(do + 1) * 512],
                                start=True,
                                stop=False,
                            )
                            nc.tensor.matmul(
                                op[:],
                                lhsT=ctxcat_sb[:, 1, s16 * 128:(s16 + 1) * 128],
                                rhs=wo_sb[:, 1, do * 512:(do + 1) * 512],
                                start=False,
                                stop=True,
                            )
                            ot = outp.tile([128, 512], F32, tag="ot", name="ot")
                            if do == 0:
                                nc.scalar.copy(out=ot[:], in_=op[:])
                            else:
                                nc.vector.tensor_copy(out=ot[:], in_=op[:])
                            nc.sync.dma_start(
                                out=out[s16 * 128:(s16 + 1) * 128,
                                        do * 512:(do + 1) * 512],
                                in_=ot[:],
                            )

                pending = None
                for qb in range(NS):
                    nch = (qb + 1) * 4
                    for h in range(HPC):
                        p, j = h // 2, h % 2
                        even = j == 0
                        qs = qt_sb[j * 64:(j + 1) * 64, p, qb * 512:(qb + 1) * 512]
                        ctx_ps = ctxp.tile([128, 512], F32, tag="ctx", name="ctx_ps")
                        ctx_out = ctx_ps[0:65, :] if even else ctx_ps[:]
                        lagged = []  # (pt, f0, diag, c) awaiting their ctx mms

                        def emit_ctx(lag):
                            pt, f0, diag, c = lag
                            lhsT_v = (
                                va_sb[:, p, c, 0:65]
                                if even
                                else va_sb[:, p, c, 64:192]
                            )
                            nc.tensor.matmul(
                                ctx_out[:, f0:512] if diag else ctx_out,
                                lhsT=lhsT_v,
                                rhs=pt[:, f0:512],
                                start=(c == 0),
                                stop=(c == nch - 1),
                            )

                        for c in range(nch):
                            st = stp.tile([128, 512], F32, tag="st", name="st")
                            diag = c >= qb * 4
                            # columns [0, f0) of this block are fully masked
                            # (q < kv for all partitions): skip them entirely.
                            f0 = 128 * (c - qb * 4) if diag else 0
                            nc.tensor.matmul(
                                st[:, f0:512],
                                lhsT=kt_sb[j * 64:(j + 1) * 64, p,
                                           c * 128:(c + 1) * 128],
                                rhs=qs[:, f0:512],
                                start=True,
                                stop=not diag,
                            )
                            if diag:
                                nc.tensor.matmul(
                                    st[:, f0:512],
                                    lhsT=ident_sb[:],
                                    rhs=mneg_sb[:, c - qb * 4, f0:512],
                                    start=False,
                                    stop=True,
                                )
                            pt = ptp.tile([128, 512], BF16, tag="pt", name="pt")
                            nc.scalar.activation(
                                out=pt[:, f0:512],
                                in_=st[:, f0:512],
                                func=mybir.ActivationFunctionType.Exp,
                            )
                            # ctx trails TWO chunks behind its exp so the PE
                            # queue stays deep (hides ACT latency, semaphore
                            # propagation, and LDWEIGHTS of the next matmul).
                            lagged.append((pt, f0, diag, c))
                            if len(lagged) > 2:
                                emit_ctx(lagged.pop(0))
                        for lag in lagged:
                            emit_ctx(lag)

                        if pending is not None:
                            emit_norm(*pending)
                        pending = (ctx_ps, even, p, qb, h)
                        if h == 0 and qb > 0:
                            emit_outproj(qb - 1)

                emit_norm(*pending)
                pending = None
                emit_outproj(NS - 1)
    if not nc.is_finalized():
        nc.finalize()
    return nc


def _prep_inputs(embeddings, Wq, bq, Wk, bk, Wv, bv, Wo, bo):
    embeddings = np.asarray(embeddings, np.float32)
    Wq, bq = np.asarray(Wq, np.float32), np.asarray(bq, np.float32)
    Wk = np.asarray(Wk, np.float32)
    Wv, bv = np.asarray(Wv, np.float32), np.asarray(bv, np.float32)
    Wo = np.asarray(Wo, np.float32)

    import ml_dtypes
    bf16_t = ml_dtypes.bfloat16
    p_idx = np.arange(128)
    mneg = np.zeros((128, 4, 512), np.float32)
    for i in range(4):
        f = np.arange(512)[None, :]
        mneg[:, i, :] = np.where(f >= p_idx[:, None] + 128 * i, 0.0, -1e9)
    mneg = mneg.astype(bf16_t)
    ident = np.eye(128, dtype=np.float32).astype(bf16_t)
    vfix = np.zeros((128, 64), np.float32)
    vfix[:, 0] = 1.0
    vfix = vfix.astype(bf16_t)
    ones = np.ones((1, 128), np.float32).astype(bf16_t)

    in_maps = []
    for c in range(NCORES):
        b, g = c // 4, c % 4
        hs = HPC * g
        xt = np.ascontiguousarray(embeddings[b].T).astype(bf16_t)
        # 1/sqrt(dk) folded into Wq/bq (exact power of two)
        wq2 = np.stack(
            [np.concatenate([Wq[hs + 2 * p], Wq[hs + 2 * p + 1]], axis=1)
             for p in range(NPAIR)]
        ) * 0.125
        wk2 = np.stack(
            [np.concatenate([Wk[hs + 2 * p], Wk[hs + 2 * p + 1]], axis=1)
             for p in range(NPAIR)]
        )
        wv4 = np.concatenate([Wv[hs + h] for h in range(HPC)], axis=1)
        wo4 = np.ascontiguousarray(Wo[hs * DK:(hs + HPC) * DK, :]).astype(bf16_t)
        bq2 = np.stack(
            [np.concatenate([bq[hs + 2 * p], bq[hs + 2 * p + 1]]) / 8.0
             for p in range(NPAIR)], axis=1
        )
        bvb = np.zeros((128, NPAIR, 2, 64), np.float32)
        for p in range(NPAIR):
            bvb[:, p, 0, :] = bv[hs + 2 * p][None, :]
            bvb[:, p, 1, :] = bv[hs + 2 * p + 1][None, :]
        in_maps.append({
            "xt": xt,
            "wq": np.ascontiguousarray(wq2).astype(bf16_t),
            "wk": np.ascontiguousarray(wk2).astype(bf16_t),
            "wv": np.ascontiguousarray(wv4).astype(bf16_t),
            "wo": wo4,
            "bq": np.ascontiguousarray(bq2),
            "bv_bc": bvb,
            "mneg": mneg,
            "ident": ident,
            "ones": ones,
            "vfix": vfix,
        })
    return in_maps


def kernel(embeddings, Wq, bq, Wk, bk, Wv, bv, Wo, bo, _trace=False, _trace_kw=None):
    if "nc" not in _CACHE:
        _CACHE["nc"] = _build_bass()
    nc = _CACHE["nc"]
    in_maps = _prep_inputs(embeddings, Wq, bq, Wk, bk, Wv, bv, Wo, bo)
    kw = dict(_trace_kw or {})
    res = run_bass_kernel_spmd(
        nc, in_maps, core_ids=list(range(NCORES)), trace=_trace, **kw
    )
    _CACHE["last_result"] = res
    bo32 = np.asarray(bo, np.float32)
    out = np.empty((B, S, D), np.float32)
    for b in range(B):
        acc = np.array(res.results[4 * b]["out"], np.float32, copy=True)
        for g in range(1, 4):
            acc += np.asarray(res.results[4 * b + g]["out"], np.float32)
        out[b] = acc + bo32
    return out
